# revision 21
# baseline (speedup 1.0000x reference)
"""Trainium2 Bass kernel for nn_Decoder_78915729096963.

Reference math (B=32, S=64, L=512, H=768, D3=128, E=768, V_SZ=30522):
  feats = emb_table[input_ids]                      # [B,S,E]
  hdc   = context @ Wh + bh                         # [B,L,D3]
  h     = tanh(concat(first_emb, first_hidden)@Wd + bd)
  for t in range(S):
      h       = tanh(concat(feats[:,t], h) @ Wd + bd)
      att_row = tanh(h@Wi + bi + hdc) . V           # [B,L]
      atts[t] = att_row[:,None,:]
      prts[t] = argmax(softmax(att_row[:,None,:],axis=1)[:,0])  # == 0 always
                # (softmax over a singleton axis is identically 1.0)

Kernel strategy:
  - Data-parallel over batch: 4 batch rows per core x 8 cores, weights
    replicated, no collectives.
  - Split Wd into Wd_top (emb rows) / Wd_bot (hidden rows):
      z[b,s] = feats[b,s]@Wd_top + bd  is precomputed as one batched matmul,
      so each sequential step only does h@Wd_bot.
  - All tensors live feature-major on SBUF partitions.  The attention tanh is
    fused as ACT(tanh, bias=q[:,b]) over hdc^T; the .V reduction is a matmul
    with V as a 1-column stationary operand.
  - Steps are software-pipelined: the V-matmul of step t-1 is emitted between
    the recurrence matmuls and the q matmul of step t so the PE never waits
    on the (slow) attention tanh.
"""

import numpy as np
import ml_dtypes

import concourse.bass as bass
import concourse.tile as tile
from concourse import bacc, mybir
from concourse.bass_utils import run_bass_kernel_spmd

# problem shapes (hardcoded; kernel.py must be self-contained)
B, S, L = 32, 64, 512
H, D3, E = 768, 128, 768
NCORES = 8
BL = B // NCORES          # batch rows per core
KC = H // 128             # contraction chunks of 128
JB = H // 128             # output-feature blocks of 128
NZ = BL * S               # precomputed-z columns per core
BLL = BL * L              # attention columns per core

F32 = mybir.dt.float32
BF16 = mybir.dt.bfloat16
AFT = mybir.ActivationFunctionType

# weight/step dtype: bf16 halves PE weight-load time (FWL) at ~3e-3 rel err
W_DT = BF16
W_NP = ml_dtypes.bfloat16


def build_nc(n_steps=S, out_dmas=True, level=4):
    nc = bacc.Bacc("TRN2", target_bir_lowering=False, debug=False)

    ctxT = nc.dram_tensor("ctxT", [H, BLL], F32, kind="ExternalInput")
    featsT = nc.dram_tensor("featsT", [H, NZ + 1], W_DT, kind="ExternalInput")
    fhT = nc.dram_tensor("fhT", [H, BL], W_DT, kind="ExternalInput")
    wdtT = nc.dram_tensor("wdtT", [H, H], W_DT, kind="ExternalInput")
    wdbT = nc.dram_tensor("wdbT", [H, H], W_DT, kind="ExternalInput")
    wiT = nc.dram_tensor("wiT", [H, D3], W_DT, kind="ExternalInput")
    whT = nc.dram_tensor("whT", [H, D3], F32, kind="ExternalInput")
    vT = nc.dram_tensor("vT", [D3, 1], W_DT, kind="ExternalInput")
    bhT = nc.dram_tensor("bhT", [D3, 1], F32, kind="ExternalInput")
    biT = nc.dram_tensor("biT", [D3, 1], F32, kind="ExternalInput")
    bdT = nc.dram_tensor("bdT", [128, JB], F32, kind="ExternalInput")
    atts = nc.dram_tensor("atts", [S, BL, L], F32, kind="ExternalOutput")

    with tile.TileContext(nc) as tc:
        _emit(tc, nc, ctxT, featsT, fhT, wdtT, wdbT, wiT, whT, vT, bhT, biT,
              bdT, atts, n_steps, out_dmas, level)
    nc.compile()
    return nc


def _emit(tc, nc, ctxT, featsT, fhT, wdtT, wdbT, wiT, whT, vT, bhT, biT, bdT,
          atts, n_steps=S, out_dmas=True, level=4):
    from contextlib import ExitStack
    ctx = ExitStack()
    with ctx:
        sb = ctx.enter_context(tc.tile_pool(name="persist", bufs=1))

        wdb_sb = sb.tile([128, KC * H], W_DT, name="wdb_sb")
        wi_sb = sb.tile([128, KC * D3], W_DT, name="wi_sb")
        v_sb = sb.tile([128, 1], W_DT, name="v_sb")
        bh_sb = sb.tile([128, 1], F32, name="bh_sb")
        bi_sb = sb.tile([128, 1], F32, name="bi_sb")
        bd_sb = sb.tile([128, JB], F32, name="bd_sb")
        fh_sb = sb.tile([128, KC, BL], W_DT, name="fh_sb")
        hdcT = sb.tile([128, BLL], F32, name="hdcT")
        zb = sb.tile([128, JB, NZ], F32, name="zb")
        zb0 = sb.tile([128, JB], F32, name="zb0")

        for k in range(KC):
            r = slice(128 * k, 128 * (k + 1))
            nc.sync.dma_start(wdb_sb[:, H * k:H * (k + 1)], wdbT[r, :])
            nc.sync.dma_start(wi_sb[:, D3 * k:D3 * (k + 1)], wiT[r, :])
            nc.sync.dma_start(fh_sb[:, k, :], fhT[r, :])
        nc.sync.dma_start(v_sb[:], vT[:, :])
        nc.sync.dma_start(bh_sb[:], bhT[:, :])
        nc.sync.dma_start(bi_sb[:], biT[:, :])
        nc.sync.dma_start(bd_sb[:], bdT[:, :])

        # ---- phase 1: hdc^T = (context@Wh)^T + bh ; z = feats@Wd_top + bd
        with tc.tile_pool(name="ph1", bufs=1) as ph1, \
             tc.tile_pool(name="ctxp", bufs=KC) as ctxp, \
             tc.tile_pool(name="pp_hdc", bufs=1, space="PSUM") as pp_hdc, \
             tc.tile_pool(name="pp_z", bufs=2, space="PSUM") as pp_z:
            wh_sb = ph1.tile([128, KC * D3], F32, name="wh_sb")
            wdt_sb = ph1.tile([128, KC * H], W_DT, name="wdt_sb")
            feats_sb = ph1.tile([128, KC, NZ + 1], W_DT, name="feats_sb")
            for k in range(KC):
                r = slice(128 * k, 128 * (k + 1))
                nc.sync.dma_start(wh_sb[:, D3 * k:D3 * (k + 1)], whT[r, :])
                nc.sync.dma_start(wdt_sb[:, H * k:H * (k + 1)], wdtT[r, :])
                nc.sync.dma_start(feats_sb[:, k, :], featsT[r, :])

            if level < 1:
                nc.sync.dma_start(atts[0, 0:1, :], wh_sb[0:1, 0:L])
                return
            hdc_ps = [pp_hdc.tile([128, 512], F32, name=f"hdc_ps{nb}")
                      for nb in range(BLL // 512)]
            for k in range(KC):
                ctx_sb = ctxp.tile([128, BLL], F32, name="ctx_sb", tag="ctx")
                nc.sync.dma_start(ctx_sb[:], ctxT[128 * k:128 * (k + 1), :])
                for nb in range(BLL // 512):
                    nc.tensor.matmul(
                        hdc_ps[nb][:],
                        lhsT=wh_sb[:, D3 * k:D3 * (k + 1)],
                        rhs=ctx_sb[:, 512 * nb:512 * (nb + 1)],
                        start=(k == 0), stop=(k == KC - 1))
            for nb in range(BLL // 512):
                nc.vector.tensor_scalar_add(
                    hdcT[:, 512 * nb:512 * (nb + 1)], hdc_ps[nb][:], bh_sb[:])

            if level < 2:
                nc.sync.dma_start(atts[0, 0:1, :], hdcT[0:1, 0:L])
                return
            for j in range(JB):
                z_ps = pp_z.tile([128, NZ + 1], F32, name="z_ps", tag="z_ps")
                for k in range(KC):
                    nc.tensor.matmul(
                        z_ps[:],
                        lhsT=wdt_sb[:, H * k + 128 * j:H * k + 128 * (j + 1)],
                        rhs=feats_sb[:, k, :],
                        start=(k == 0), stop=(k == KC - 1))
                nc.vector.tensor_scalar_add(zb[:, j, :], z_ps[:, :NZ],
                                            bd_sb[:, j:j + 1])
                nc.vector.tensor_scalar_add(zb0[:, j:j + 1], z_ps[:, NZ:NZ + 1],
                                            bd_sb[:, j:j + 1])

        # ---- phase 2: the 64-step recurrence
        hp = ctx.enter_context(tc.tile_pool(name="hpool", bufs=2))
        apool = ctx.enter_context(tc.tile_pool(name="apool", bufs=2))
        php = ctx.enter_context(tc.tile_pool(name="php", bufs=2, space="PSUM"))
        pqp = ctx.enter_context(tc.tile_pool(name="pqp", bufs=2, space="PSUM"))
        pap = ctx.enter_context(tc.tile_pool(name="pap", bufs=4, space="PSUM"))
        stgp = ctx.enter_context(tc.tile_pool(name="stgp", bufs=8))

        def advance(h_rhs, t):
            """h_new = tanh(Wd_bot^T . h + z[t]); t==-1 uses first_* inputs."""
            ph = php.tile([128, JB, BL], F32, name="ph", tag="ph")
            for j in range(JB):
                for k in range(KC):
                    nc.tensor.matmul(
                        ph[:, j, :],
                        lhsT=wdb_sb[:, H * k + 128 * j:H * k + 128 * (j + 1)],
                        rhs=h_rhs[:, k, :],
                        start=(k == 0), stop=(k == KC - 1))
            hpre = hp.tile([128, JB, BL], F32, name="hpre", tag="hpre")
            if t < 0:
                for j in range(JB):
                    nc.vector.tensor_scalar_add(hpre[:, j, :], ph[:, j, :],
                                                zb0[:, j:j + 1])
            else:
                nc.vector.tensor_add(hpre[:], ph[:], zb[:, :, BL * t:BL * (t + 1)])
            hn = hp.tile([128, KC, BL], W_DT, name="hn", tag="hn")
            nc.scalar.activation(hn[:], hpre[:], AFT.Tanh)
            return hn

        def emit_att(t, a_prev):
            for b in range(BL):
                pa = pap.tile([1, L], F32, name="pa", tag="pa")
                nc.tensor.matmul(pa[:], lhsT=v_sb[:, 0:1],
                                 rhs=a_prev[:, L * b:L * (b + 1)],
                                 start=True, stop=True)
                stg = stgp.tile([1, L], F32, name="stg", tag="stg")
                nc.vector.tensor_copy(stg[0:1, :], pa[0:1, :])
                if out_dmas:
                    nc.sync.dma_start(atts[t, b:b + 1, :], stg[0:1, :])

        if level < 3:
            nc.sync.dma_start(atts[0, 0:1, :], zb[0:1, 0, 0:L])
            return
        h_cur = advance(fh_sb, -1)
        if level < 4:
            nc.sync.dma_start(atts[0, 0:1, :], zb[0:1, 0, 0:L])
            return
        a_prev = None
        for t in range(n_steps):
            h_next = advance(h_cur, t)
            if a_prev is not None:
                emit_att(t - 1, a_prev)
            pq = pqp.tile([128, BL], F32, name="pq", tag="pq")
            for k in range(KC):
                nc.tensor.matmul(pq[:], lhsT=wi_sb[:, D3 * k:D3 * (k + 1)],
                                 rhs=h_next[:, k, :],
                                 start=(k == 0), stop=(k == KC - 1))
            q_sb = hp.tile([128, BL], F32, name="q_sb", tag="q_sb")
            nc.vector.tensor_scalar_add(q_sb[:], pq[:], bi_sb[:])
            a_sb = apool.tile([128, BLL], W_DT, name="a_sb", tag="a_sb")
            for b in range(BL):
                nc.scalar.activation(a_sb[:, L * b:L * (b + 1)],
                                     hdcT[:, L * b:L * (b + 1)],
                                     AFT.Tanh, bias=q_sb[:, b:b + 1])
            a_prev = a_sb
            h_cur = h_next
        emit_att(n_steps - 1, a_prev)
        if not out_dmas:
            nc.sync.dma_start(atts[0, 0:1, :], hdcT[0:1, 0:L])


def make_in_maps(input_ids, first_hidden, context, emb_table, Wi, bi, Wh, bh,
                 V, first_emb, Wd, bd):
    """Host-side sharding + layout prep. All args are np.ndarray (fp32/int)."""
    feats = emb_table[input_ids.astype(np.int64)]        # [B, S, E]
    wdtT = np.ascontiguousarray(Wd[:E, :]).astype(W_NP)
    wdbT = np.ascontiguousarray(Wd[E:, :]).astype(W_NP)
    wiT = Wi.astype(W_NP)
    whT = np.ascontiguousarray(Wh.astype(np.float32))
    vT = V.reshape(D3, 1).astype(W_NP)
    bhT = bh.reshape(D3, 1).astype(np.float32)
    biT = bi.reshape(D3, 1).astype(np.float32)
    bdT = np.ascontiguousarray(bd.reshape(JB, 128).T).astype(np.float32)

    in_maps = []
    for c in range(NCORES):
        b0 = c * BL
        ctxT = np.ascontiguousarray(
            context[b0:b0 + BL].transpose(2, 0, 1).reshape(H, BLL)
        ).astype(np.float32)
        featsT = np.empty((H, NZ + 1), dtype=W_NP)
        # column order: s*BL + b  (so step t's batch columns are contiguous)
        featsT[:, :NZ] = feats[b0:b0 + BL].transpose(2, 1, 0).reshape(H, NZ)
        featsT[:, NZ] = first_emb
        fhT = np.ascontiguousarray(first_hidden[b0:b0 + BL].T).astype(W_NP)
        in_maps.append(dict(ctxT=ctxT, featsT=featsT, fhT=fhT, wdtT=wdtT,
                            wdbT=wdbT, wiT=wiT, whT=whT, vT=vT, bhT=bhT,
                            biT=biT, bdT=bdT))
    return in_maps


_NC_CACHE = []


def kernel(input_ids, mask, first_hidden, context, emb_table, Wi, bi, Wh, bh,
           V, first_emb, Wd, bd):
    input_ids = np.asarray(input_ids)
    first_hidden = np.asarray(first_hidden, dtype=np.float32)
    context = np.asarray(context, dtype=np.float32)
    emb_table = np.asarray(emb_table, dtype=np.float32)
    Wi = np.asarray(Wi, dtype=np.float32)
    bi = np.asarray(bi, dtype=np.float32)
    Wh = np.asarray(Wh, dtype=np.float32)
    bh = np.asarray(bh, dtype=np.float32)
    V = np.asarray(V, dtype=np.float32)
    first_emb = np.asarray(first_emb, dtype=np.float32)
    Wd = np.asarray(Wd, dtype=np.float32)
    bd = np.asarray(bd, dtype=np.float32)

    if not _NC_CACHE:
        _NC_CACHE.append(build_nc())
    nc = _NC_CACHE[0]

    in_maps = make_in_maps(input_ids, first_hidden, context, emb_table, Wi,
                           bi, Wh, bh, V, first_emb, Wd, bd)
    res = run_bass_kernel_spmd(nc, in_maps, core_ids=list(range(NCORES)))

    atts = np.empty((S, B, 1, L), dtype=np.float32)
    for c in range(NCORES):
        atts[:, c * BL:(c + 1) * BL, 0, :] = res.results[c]["atts"]
    prts = np.zeros((S, B), dtype=np.int32)
    return atts, prts


# revision 47
# speedup vs baseline: 10383.2748x; 10383.2748x over previous
"""Trainium2 Bass kernel for nn_Decoder_78915729096963.

Reference math (B=32, S=64, L=512, H=768, D3=128, E=768, V_SZ=30522):
  feats = emb_table[input_ids]                      # [B,S,E]
  hdc   = context @ Wh + bh                         # [B,L,D3]
  h     = tanh(concat(first_emb, first_hidden)@Wd + bd)
  for t in range(S):
      h       = tanh(concat(feats[:,t], h) @ Wd + bd)
      att_row = tanh(h@Wi + bi + hdc) . V           # [B,L]
      atts[t] = att_row[:,None,:]
      prts[t] = argmax(softmax(att_row[:,None,:],axis=1)[:,0])  # == 0 always
                # (softmax over a singleton axis is identically 1.0)

Kernel strategy:
  - Data-parallel over batch: 4 batch rows per core x 8 cores, weights
    replicated, no collectives.
  - Split Wd into Wd_top (emb rows) / Wd_bot (hidden rows):
      z[b,s] = feats[b,s]@Wd_top + bd  is precomputed as one batched matmul,
      so each sequential step only does h@Wd_bot.
  - All tensors live feature-major on SBUF partitions.  The attention tanh is
    fused as ACT(tanh, bias=q[:,b]) over hdc^T; the .V reduction is a matmul
    with V as a 1-column stationary operand.
  - Steps are software-pipelined: the V-matmul of step t-1 is emitted between
    the recurrence matmuls and the q matmul of step t so the PE never waits
    on the (slow) attention tanh.
"""

import numpy as np
import ml_dtypes

import concourse.bass as bass
import concourse.tile as tile
from concourse import bacc, mybir
from concourse.bass_utils import run_bass_kernel_spmd

# problem shapes (hardcoded; kernel.py must be self-contained)
B, S, L = 32, 64, 512
H, D3, E = 768, 128, 768
NCORES = 8
BL = B // NCORES          # batch rows per core
KC = H // 128             # contraction chunks of 128
JB = H // 128             # output-feature blocks of 128
NZ = BL * S               # precomputed-z columns per core
BLL = BL * L              # attention columns per core

F32 = mybir.dt.float32
F32R = mybir.dt.float32r     # fp32 bits, 1 cyc/row matmul (vs 4 for fp32)
BF16 = mybir.dt.bfloat16
AFT = mybir.ActivationFunctionType

# weight/step dtype: bf16 halves PE weight-load time (FWL) at ~3e-3 rel err
W_DT = BF16
W_NP = ml_dtypes.bfloat16


def build_nc(n_steps=S, out_dmas=True, level=4):
    nc = bacc.Bacc("TRN2", target_bir_lowering=False, debug=False)

    ctxT = nc.dram_tensor("ctxT", [H, BLL], F32R, kind="ExternalInput")
    featsT = nc.dram_tensor("featsT", [H, NZ + 1], W_DT, kind="ExternalInput")
    fhT = nc.dram_tensor("fhT", [H, BL], W_DT, kind="ExternalInput")
    wdtT = nc.dram_tensor("wdtT", [H, H], W_DT, kind="ExternalInput")
    wdbT = nc.dram_tensor("wdbT", [H, H], W_DT, kind="ExternalInput")
    wiT = nc.dram_tensor("wiT", [H, D3], W_DT, kind="ExternalInput")
    whT = nc.dram_tensor("whT", [H, D3], F32R, kind="ExternalInput")
    vT = nc.dram_tensor("vT", [D3, 1], W_DT, kind="ExternalInput")
    bhT = nc.dram_tensor("bhT", [D3, 1], F32, kind="ExternalInput")
    biT = nc.dram_tensor("biT", [D3, 1], F32, kind="ExternalInput")
    bdT = nc.dram_tensor("bdT", [128, JB], F32, kind="ExternalInput")
    atts = nc.dram_tensor("atts", [S, BL, L], F32, kind="ExternalOutput")

    with tile.TileContext(nc) as tc:
        _emit(tc, nc, ctxT, featsT, fhT, wdtT, wdbT, wiT, whT, vT, bhT, biT,
              bdT, atts, n_steps, out_dmas, level)
    nc.compile()
    return nc


def _emit(tc, nc, ctxT, featsT, fhT, wdtT, wdbT, wiT, whT, vT, bhT, biT, bdT,
          atts, n_steps=S, out_dmas=True, level=4):
    from contextlib import ExitStack
    ctx = ExitStack()
    with ctx:
        sb = ctx.enter_context(tc.tile_pool(name="persist", bufs=1))

        wdb_sb = sb.tile([128, KC * H], W_DT, name="wdb_sb")
        wi_sb = sb.tile([128, KC * D3], W_DT, name="wi_sb")
        # V in col 3 of a 36-wide tile: window [3-s : 35-s] puts V at
        # relative column s, so step s of a 4-step group lands its V-matmul
        # result on psum partition 32b+s (other rows accumulate zeros)
        v_sb = sb.tile([128, 36], W_DT, name="v_sb")
        bh_sb = sb.tile([128, 1], F32, name="bh_sb")
        bi_sb = sb.tile([128, 1], F32, name="bi_sb")
        bd_sb = sb.tile([128, JB], F32, name="bd_sb")
        fh_sb = sb.tile([128, KC, BL], W_DT, name="fh_sb")
        hdcT = sb.tile([128, BLL], F32, name="hdcT")
        zb = sb.tile([128, JB, NZ], F32, name="zb")
        zb0 = sb.tile([128, JB], F32, name="zb0")

        def load_persistent():
            nc.sync.dma_start(wdb_sb.rearrange("p (k h) -> p k h", k=KC),
                              wdbT.ap().rearrange("(k p) h -> p k h", p=128))
            nc.sync.dma_start(wi_sb.rearrange("p (k d) -> p k d", k=KC),
                              wiT.ap().rearrange("(k p) d -> p k d", p=128))
            nc.sync.dma_start(fh_sb[:, :, :],
                              fhT.ap().rearrange("(k p) b -> p k b", p=128))
            nc.vector.memset(v_sb[:], 0.0)
            nc.sync.dma_start(v_sb[:, 3:4], vT[:, :])
            nc.sync.dma_start(bh_sb[:], bhT[:, :])
            nc.sync.dma_start(bi_sb[:], biT[:, :])
            nc.sync.dma_start(bd_sb[:], bdT[:, :])

        # phase-2 SBUF pools are created BEFORE phase 1 so their addresses
        # don't alias the released phase-1 pools (the sim's conflict checker
        # is conservative about partition-strided DMA reads of reused space)
        hp = ctx.enter_context(tc.tile_pool(name="hpool", bufs=2))
        apool = ctx.enter_context(tc.tile_pool(name="apool", bufs=2))
        stgp = ctx.enter_context(tc.tile_pool(name="stgp", bufs=2))

        # ---- phase 1: hdc^T = (context@Wh)^T + bh ; z = feats@Wd_top + bd
        with tc.tile_pool(name="ph1", bufs=1) as ph1, \
             tc.tile_pool(name="ctxp", bufs=KC) as ctxp, \
             tc.tile_pool(name="pp_hdc", bufs=1, space="PSUM") as pp_hdc, \
             tc.tile_pool(name="pp_z", bufs=2, space="PSUM") as pp_z:
            wh_sb = ph1.tile([128, KC * D3], F32R, name="wh_sb")
            wdt_sb = ph1.tile([128, KC * H], W_DT, name="wdt_sb")
            feats_sb = ph1.tile([128, KC, NZ + 1], W_DT, name="feats_sb")
            nc.sync.dma_start(wh_sb.rearrange("p (k d) -> p k d", k=KC),
                              whT.ap().rearrange("(k p) d -> p k d", p=128))
            nc.sync.dma_start(wdt_sb.rearrange("p (k h) -> p k h", k=KC),
                              wdtT.ap().rearrange("(k p) h -> p k h", p=128))
            nc.sync.dma_start(feats_sb[:, :, :],
                              featsT.ap().rearrange("(k p) n -> p k n", p=128))

            if level < 1:
                nc.sync.dma_start(atts[0, 0:1, :], wh_sb[0:1, 0:L])
                return
            hdc_ps = [pp_hdc.tile([128, 512], F32, name=f"hdc_ps{nb}")
                      for nb in range(BLL // 512)]
            for k in range(KC):
                ctx_sb = ctxp.tile([128, BLL], F32R, name="ctx_sb", tag="ctx")
                nc.sync.dma_start(ctx_sb[:], ctxT[128 * k:128 * (k + 1), :])
                for nb in range(BLL // 512):
                    nc.tensor.matmul(
                        hdc_ps[nb][:],
                        lhsT=wh_sb[:, D3 * k:D3 * (k + 1)],
                        rhs=ctx_sb[:, 512 * nb:512 * (nb + 1)],
                        start=(k == 0), stop=(k == KC - 1))
                if k == 0:
                    # issue after the critical ctx/feats loads are queued
                    load_persistent()
            for nb in range(BLL // 512):
                # fold both bh and bi into hdcT (bi would otherwise be added
                # to q every step)
                nc.vector.tensor_scalar(
                    hdcT[:, 512 * nb:512 * (nb + 1)], hdc_ps[nb][:],
                    bh_sb[:], bi_sb[:],
                    mybir.AluOpType.add, mybir.AluOpType.add)

            if level < 2:
                nc.sync.dma_start(atts[0, 0:1, :], hdcT[0:1, 0:L])
                return
            for j in range(JB):
                z_ps = pp_z.tile([128, NZ + 1], F32, name="z_ps", tag="z_ps")
                for k in range(KC):
                    nc.tensor.matmul(
                        z_ps[:],
                        lhsT=wdt_sb[:, H * k + 128 * j:H * k + 128 * (j + 1)],
                        rhs=feats_sb[:, k, :],
                        start=(k == 0), stop=(k == KC - 1))
                nc.vector.tensor_scalar_add(zb[:, j, :], z_ps[:, :NZ],
                                            bd_sb[:, j:j + 1])
                nc.vector.tensor_scalar_add(zb0[:, j:j + 1], z_ps[:, NZ:NZ + 1],
                                            bd_sb[:, j:j + 1])

        # ---- phase 2: the 64-step recurrence
        php = ctx.enter_context(tc.tile_pool(name="php", bufs=2, space="PSUM"))
        pqp = ctx.enter_context(tc.tile_pool(name="pqp", bufs=2, space="PSUM"))
        pap = ctx.enter_context(tc.tile_pool(name="pap", bufs=2, space="PSUM"))
        dscr = ctx.enter_context(tc.tile_pool(name="dscr", bufs=2,
                                              space="DRAM"))
        GRP = 4                       # steps per output psum/DMA group

        def advance(h_rhs, t):
            """h_new = tanh(Wd_bot^T . h + z[t]); t==-1 uses first_* inputs."""
            ph = php.tile([128, JB, BL], F32, name="ph", tag="ph")
            for j in range(JB):
                for k in range(KC):
                    nc.tensor.matmul(
                        ph[:, j, :],
                        lhsT=wdb_sb[:, H * k + 128 * j:H * k + 128 * (j + 1)],
                        rhs=h_rhs[:, k, :],
                        start=(k == 0), stop=(k == KC - 1))
            hpre = hp.tile([128, JB, BL], F32, name="hpre", tag="hpre")
            if t < 0:
                for j in range(JB):
                    nc.vector.tensor_scalar_add(hpre[:, j, :], ph[:, j, :],
                                                zb0[:, j:j + 1])
            else:
                nc.vector.tensor_add(hpre[:], ph[:], zb[:, :, BL * t:BL * (t + 1)])
            hn = hp.tile([128, KC, BL], W_DT, name="hn", tag="hn")
            nc.scalar.activation(hn[:], hpre[:], AFT.Tanh)
            return hn

        pa_cur = [None]

        def emit_att(t, a_prev, last):
            # 4 col-tiled V-matmuls; step s=t%4 of the group lands on psum
            # partition 32b+s (V sits at relative column s of the window).
            # Rows written by other steps accumulate zeros.
            g, s = divmod(t, GRP)
            if s == 0:
                pa_cur[0] = pap.tile([128, L], F32, name="pa", tag="pa")
            pa = pa_cur[0]
            for b in range(BL):
                nc.tensor.matmul(pa[32 * b:32 * (b + 1), :],
                                 lhsT=v_sb[:, 3 - s:35 - s],
                                 rhs=a_prev[:, L * b:L * (b + 1)],
                                 start=(s == 0), stop=(s == GRP - 1 or last),
                                 tile_position=(0, 32 * b),
                                 skip_group_check=True)
            if s == GRP - 1 or last:
                stg = stgp.tile([128, L], F32, name="stg", tag="stg")
                nc.vector.tensor_copy(stg[:], pa[:])
                if out_dmas:
                    # bounce through DRAM: SBUF->DRAM contiguous, then a
                    # DRAM->DRAM gather of rows {32b+i} -> atts[4g+i, b, :]
                    scr = dscr.tile([128, L], F32, name="scr", tag="scr")
                    nc.sync.dma_start(scr[:, :], stg[:])
                    src = scr.rearrange("(b i) l -> b i l", b=BL)[:, 0:s + 1, :]
                    out = atts[GRP * g:GRP * g + s + 1, :, :].rearrange(
                        "s b l -> b s l")
                    nc.sync.dma_start(out, src)

        if level < 3:
            nc.sync.dma_start(atts[0, 0:1, :], zb[0:1, 0, 0:L])
            return
        h_cur = advance(fh_sb, -1)                      # h(0)
        if level < 4:
            nc.sync.dma_start(atts[0, 0:1, :], zb[0:1, 0, 0:L])
            return
        h_nxt = advance(h_cur, 0)                       # h(1)
        a_prev = None
        # Steady state of iteration t: q(t) from h(t+1); the 4 attention
        # tanhs of step t are split around the (cheap, critical) h(t+2)
        # recurrence ops so ACT never idles and the recurrence never queues
        # behind a full attention burst.  V-matmuls run one step behind so
        # the PE never waits on ACT.
        for t in range(n_steps):
            pq = pqp.tile([128, BL], F32, name="pq", tag="pq")
            for k in range(KC):
                nc.tensor.matmul(pq[:], lhsT=wi_sb[:, D3 * k:D3 * (k + 1)],
                                 rhs=h_nxt[:, k, :],
                                 start=(k == 0), stop=(k == KC - 1))
            # pre-add q (per-b broadcast, read straight from psum) on DVE so
            # the attention tanh is one big ACT op instead of four biased
            # ones; bi is already folded into hdcT
            apre = apool.tile([128, BLL], F32, name="apre", tag="apre")
            for b in range(BL):
                nc.vector.tensor_scalar_add(apre[:, L * b:L * (b + 1)],
                                            hdcT[:, L * b:L * (b + 1)],
                                            pq[:, b:b + 1])
            a_sb = apool.tile([128, BLL], W_DT, name="a_sb", tag="a_sb")
            nc.scalar.activation(a_sb[:, 0:2 * L], apre[:, 0:2 * L], AFT.Tanh)
            h_new = advance(h_nxt, t + 1) if t + 1 < n_steps else None
            nc.scalar.activation(a_sb[:, 2 * L:BLL], apre[:, 2 * L:BLL],
                                 AFT.Tanh)
            if a_prev is not None:
                emit_att(t - 1, a_prev, last=False)
            a_prev = a_sb
            h_cur, h_nxt = h_nxt, h_new
        emit_att(n_steps - 1, a_prev, last=True)
        if not out_dmas:
            nc.sync.dma_start(atts[0, 0:1, :], hdcT[0:1, 0:L])


def make_in_maps(input_ids, first_hidden, context, emb_table, Wi, bi, Wh, bh,
                 V, first_emb, Wd, bd):
    """Host-side sharding + layout prep. All args are np.ndarray (fp32/int)."""
    feats = emb_table[input_ids.astype(np.int64)]        # [B, S, E]
    wdtT = np.ascontiguousarray(Wd[:E, :]).astype(W_NP)
    wdbT = np.ascontiguousarray(Wd[E:, :]).astype(W_NP)
    wiT = Wi.astype(W_NP)
    whT = np.ascontiguousarray(Wh.astype(np.float32))
    vT = V.reshape(D3, 1).astype(W_NP)
    bhT = bh.reshape(D3, 1).astype(np.float32)
    biT = bi.reshape(D3, 1).astype(np.float32)
    bdT = np.ascontiguousarray(bd.reshape(JB, 128).T).astype(np.float32)

    in_maps = []
    for c in range(NCORES):
        b0 = c * BL
        ctxT = np.ascontiguousarray(
            context[b0:b0 + BL].transpose(2, 0, 1).reshape(H, BLL)
        ).astype(np.float32)
        featsT = np.empty((H, NZ + 1), dtype=W_NP)
        # column order: s*BL + b  (so step t's batch columns are contiguous)
        featsT[:, :NZ] = feats[b0:b0 + BL].transpose(2, 1, 0).reshape(H, NZ)
        featsT[:, NZ] = first_emb
        fhT = np.ascontiguousarray(first_hidden[b0:b0 + BL].T).astype(W_NP)
        in_maps.append(dict(ctxT=ctxT, featsT=featsT, fhT=fhT, wdtT=wdtT,
                            wdbT=wdbT, wiT=wiT, whT=whT, vT=vT, bhT=bhT,
                            biT=biT, bdT=bdT))
    return in_maps


_NC_CACHE = []


def kernel(input_ids, mask, first_hidden, context, emb_table, Wi, bi, Wh, bh,
           V, first_emb, Wd, bd):
    input_ids = np.asarray(input_ids)
    first_hidden = np.asarray(first_hidden, dtype=np.float32)
    context = np.asarray(context, dtype=np.float32)
    emb_table = np.asarray(emb_table, dtype=np.float32)
    Wi = np.asarray(Wi, dtype=np.float32)
    bi = np.asarray(bi, dtype=np.float32)
    Wh = np.asarray(Wh, dtype=np.float32)
    bh = np.asarray(bh, dtype=np.float32)
    V = np.asarray(V, dtype=np.float32)
    first_emb = np.asarray(first_emb, dtype=np.float32)
    Wd = np.asarray(Wd, dtype=np.float32)
    bd = np.asarray(bd, dtype=np.float32)

    if not _NC_CACHE:
        _NC_CACHE.append(build_nc())
    nc = _NC_CACHE[0]

    in_maps = make_in_maps(input_ids, first_hidden, context, emb_table, Wi,
                           bi, Wh, bh, V, first_emb, Wd, bd)
    res = run_bass_kernel_spmd(nc, in_maps, core_ids=list(range(NCORES)))

    atts = np.empty((S, B, 1, L), dtype=np.float32)
    for c in range(NCORES):
        atts[:, c * BL:(c + 1) * BL, 0, :] = res.results[c]["atts"]
    prts = np.zeros((S, B), dtype=np.int32)
    return atts, prts


# revision 52
# speedup vs baseline: 10395.3681x; 1.0012x over previous
"""Trainium2 Bass kernel for nn_Decoder_78915729096963.

Reference math (B=32, S=64, L=512, H=768, D3=128, E=768, V_SZ=30522):
  feats = emb_table[input_ids]                      # [B,S,E]
  hdc   = context @ Wh + bh                         # [B,L,D3]
  h     = tanh(concat(first_emb, first_hidden)@Wd + bd)
  for t in range(S):
      h       = tanh(concat(feats[:,t], h) @ Wd + bd)
      att_row = tanh(h@Wi + bi + hdc) . V           # [B,L]
      atts[t] = att_row[:,None,:]
      prts[t] = argmax(softmax(att_row[:,None,:],axis=1)[:,0])  # == 0 always
                # (softmax over a singleton axis is identically 1.0)

Kernel strategy:
  - Data-parallel over batch: 4 batch rows per core x 8 cores, weights
    replicated, no collectives.
  - Split Wd into Wd_top (emb rows) / Wd_bot (hidden rows):
      z[b,s] = feats[b,s]@Wd_top + bd  is precomputed as one batched matmul,
      so each sequential step only does h@Wd_bot.
  - All tensors live feature-major on SBUF partitions.  The attention tanh is
    fused as ACT(tanh, bias=q[:,b]) over hdc^T; the .V reduction is a matmul
    with V as a 1-column stationary operand.
  - Steps are software-pipelined: the V-matmul of step t-1 is emitted between
    the recurrence matmuls and the q matmul of step t so the PE never waits
    on the (slow) attention tanh.
"""

import numpy as np
import ml_dtypes

import concourse.bass as bass
import concourse.tile as tile
from concourse import bacc, mybir
from concourse.bass_utils import run_bass_kernel_spmd

# problem shapes (hardcoded; kernel.py must be self-contained)
B, S, L = 32, 64, 512
H, D3, E = 768, 128, 768
NCORES = 8
BL = B // NCORES          # batch rows per core
KC = H // 128             # contraction chunks of 128
JB = H // 128             # output-feature blocks of 128
NZ = BL * S               # precomputed-z columns per core
BLL = BL * L              # attention columns per core

F32 = mybir.dt.float32
F32R = mybir.dt.float32r     # fp32 bits, 1 cyc/row matmul (vs 4 for fp32)
BF16 = mybir.dt.bfloat16
AFT = mybir.ActivationFunctionType

# weight/step dtype: bf16 halves PE weight-load time (FWL) at ~3e-3 rel err
W_DT = BF16
W_NP = ml_dtypes.bfloat16


def build_nc(n_steps=S, out_dmas=True, level=4):
    nc = bacc.Bacc("TRN2", target_bir_lowering=False, debug=False)

    ctxT = nc.dram_tensor("ctxT", [H, BLL], F32R, kind="ExternalInput")
    featsT = nc.dram_tensor("featsT", [H, NZ + 1], W_DT, kind="ExternalInput")
    fhT = nc.dram_tensor("fhT", [H, BL], W_DT, kind="ExternalInput")
    wdtT = nc.dram_tensor("wdtT", [H, H], W_DT, kind="ExternalInput")
    wdbT = nc.dram_tensor("wdbT", [H, H], W_DT, kind="ExternalInput")
    wiT = nc.dram_tensor("wiT", [H, D3], W_DT, kind="ExternalInput")
    whT = nc.dram_tensor("whT", [H, D3], F32R, kind="ExternalInput")
    vT = nc.dram_tensor("vT", [D3, 1], W_DT, kind="ExternalInput")
    bhT = nc.dram_tensor("bhT", [D3, 1], F32, kind="ExternalInput")
    biT = nc.dram_tensor("biT", [D3, 1], F32, kind="ExternalInput")
    bdT = nc.dram_tensor("bdT", [128, JB], F32, kind="ExternalInput")
    atts = nc.dram_tensor("atts", [S, BL, L], F32, kind="ExternalOutput")

    with tile.TileContext(nc) as tc:
        _emit(tc, nc, ctxT, featsT, fhT, wdtT, wdbT, wiT, whT, vT, bhT, biT,
              bdT, atts, n_steps, out_dmas, level)
    nc.compile()
    return nc


def _emit(tc, nc, ctxT, featsT, fhT, wdtT, wdbT, wiT, whT, vT, bhT, biT, bdT,
          atts, n_steps=S, out_dmas=True, level=4):
    from contextlib import ExitStack
    ctx = ExitStack()
    with ctx:
        sb = ctx.enter_context(tc.tile_pool(name="persist", bufs=1))

        wdb_sb = sb.tile([128, KC * H], W_DT, name="wdb_sb")
        wi_sb = sb.tile([128, KC * D3], W_DT, name="wi_sb")
        # V in col 3 of a 36-wide tile: window [3-s : 35-s] puts V at
        # relative column s, so step s of a 4-step group lands its V-matmul
        # result on psum partition 32b+s (other rows accumulate zeros)
        v_sb = sb.tile([128, 36], W_DT, name="v_sb")
        bh_sb = sb.tile([128, 1], F32, name="bh_sb")
        bi_sb = sb.tile([128, 1], F32, name="bi_sb")
        bd_sb = sb.tile([128, JB], F32, name="bd_sb")
        fh_sb = sb.tile([128, KC, BL], W_DT, name="fh_sb")
        hdcT = sb.tile([128, BLL], F32, name="hdcT")
        zb = sb.tile([128, JB, NZ], F32, name="zb")
        zb0 = sb.tile([128, JB], F32, name="zb0")

        def load_persistent():
            nc.sync.dma_start(wdb_sb.rearrange("p (k h) -> p k h", k=KC),
                              wdbT.ap().rearrange("(k p) h -> p k h", p=128))
            nc.sync.dma_start(wi_sb.rearrange("p (k d) -> p k d", k=KC),
                              wiT.ap().rearrange("(k p) d -> p k d", p=128))
            nc.sync.dma_start(fh_sb[:, :, :],
                              fhT.ap().rearrange("(k p) b -> p k b", p=128))
            nc.vector.memset(v_sb[:], 0.0)
            nc.sync.dma_start(v_sb[:, 3:4], vT[:, :])
            nc.sync.dma_start(bh_sb[:], bhT[:, :])
            nc.sync.dma_start(bi_sb[:], biT[:, :])
            nc.sync.dma_start(bd_sb[:], bdT[:, :])

        # phase-2 SBUF pools are created BEFORE phase 1 so their addresses
        # don't alias the released phase-1 pools (the sim's conflict checker
        # is conservative about partition-strided DMA reads of reused space)
        hp = ctx.enter_context(tc.tile_pool(name="hpool", bufs=3))
        apool = ctx.enter_context(tc.tile_pool(name="apool", bufs=3))
        stgp = ctx.enter_context(tc.tile_pool(name="stgp", bufs=2))

        # ---- phase 1: hdc^T = (context@Wh)^T + bh ; z = feats@Wd_top + bd
        with tc.tile_pool(name="ph1", bufs=1) as ph1, \
             tc.tile_pool(name="ctxp", bufs=KC) as ctxp, \
             tc.tile_pool(name="pp_hdc", bufs=1, space="PSUM") as pp_hdc, \
             tc.tile_pool(name="pp_z", bufs=2, space="PSUM") as pp_z:
            wh_sb = ph1.tile([128, KC * D3], F32R, name="wh_sb")
            wdt_sb = ph1.tile([128, KC * H], W_DT, name="wdt_sb")
            feats_sb = ph1.tile([128, KC, NZ + 1], W_DT, name="feats_sb")
            nc.sync.dma_start(wh_sb.rearrange("p (k d) -> p k d", k=KC),
                              whT.ap().rearrange("(k p) d -> p k d", p=128))
            nc.sync.dma_start(wdt_sb.rearrange("p (k h) -> p k h", k=KC),
                              wdtT.ap().rearrange("(k p) h -> p k h", p=128))
            nc.sync.dma_start(feats_sb[:, :, :],
                              featsT.ap().rearrange("(k p) n -> p k n", p=128))

            if level < 1:
                nc.sync.dma_start(atts[0, 0:1, :], wh_sb[0:1, 0:L])
                return
            hdc_ps = [pp_hdc.tile([128, 512], F32, name=f"hdc_ps{nb}")
                      for nb in range(BLL // 512)]
            for k in range(KC):
                ctx_sb = ctxp.tile([128, BLL], F32R, name="ctx_sb", tag="ctx")
                nc.sync.dma_start(ctx_sb[:], ctxT[128 * k:128 * (k + 1), :])
                for nb in range(BLL // 512):
                    nc.tensor.matmul(
                        hdc_ps[nb][:],
                        lhsT=wh_sb[:, D3 * k:D3 * (k + 1)],
                        rhs=ctx_sb[:, 512 * nb:512 * (nb + 1)],
                        start=(k == 0), stop=(k == KC - 1))
                if k == 0:
                    # issue after the critical ctx/feats loads are queued
                    load_persistent()
            for nb in range(BLL // 512):
                # fold both bh and bi into hdcT (bi would otherwise be added
                # to q every step)
                nc.vector.tensor_scalar(
                    hdcT[:, 512 * nb:512 * (nb + 1)], hdc_ps[nb][:],
                    bh_sb[:], bi_sb[:],
                    mybir.AluOpType.add, mybir.AluOpType.add)

            if level < 2:
                nc.sync.dma_start(atts[0, 0:1, :], hdcT[0:1, 0:L])
                return
            for j in range(JB):
                z_ps = pp_z.tile([128, NZ + 1], F32, name="z_ps", tag="z_ps")
                for k in range(KC):
                    nc.tensor.matmul(
                        z_ps[:],
                        lhsT=wdt_sb[:, H * k + 128 * j:H * k + 128 * (j + 1)],
                        rhs=feats_sb[:, k, :],
                        start=(k == 0), stop=(k == KC - 1))
                nc.vector.tensor_scalar_add(zb[:, j, :], z_ps[:, :NZ],
                                            bd_sb[:, j:j + 1])
                nc.vector.tensor_scalar_add(zb0[:, j:j + 1], z_ps[:, NZ:NZ + 1],
                                            bd_sb[:, j:j + 1])

        # ---- phase 2: the 64-step recurrence
        php = ctx.enter_context(tc.tile_pool(name="php", bufs=2, space="PSUM"))
        pqp = ctx.enter_context(tc.tile_pool(name="pqp", bufs=4, space="PSUM"))
        pap = ctx.enter_context(tc.tile_pool(name="pap", bufs=2, space="PSUM"))
        dscr = ctx.enter_context(tc.tile_pool(name="dscr", bufs=2,
                                              space="DRAM"))
        GRP = 4                       # steps per output psum/DMA group

        def advance(h_rhs, t):
            """h_new = tanh(Wd_bot^T . h + z[t]); t==-1 uses first_* inputs."""
            ph = php.tile([128, JB, BL], F32, name="ph", tag="ph")
            for j in range(JB):
                for k in range(KC):
                    nc.tensor.matmul(
                        ph[:, j, :],
                        lhsT=wdb_sb[:, H * k + 128 * j:H * k + 128 * (j + 1)],
                        rhs=h_rhs[:, k, :],
                        start=(k == 0), stop=(k == KC - 1))
            hpre = hp.tile([128, JB, BL], F32, name="hpre", tag="hpre")
            if t < 0:
                for j in range(JB):
                    nc.vector.tensor_scalar_add(hpre[:, j, :], ph[:, j, :],
                                                zb0[:, j:j + 1])
            else:
                nc.vector.tensor_add(hpre[:], ph[:], zb[:, :, BL * t:BL * (t + 1)])
            hn = hp.tile([128, KC, BL], W_DT, name="hn", tag="hn")
            nc.scalar.activation(hn[:], hpre[:], AFT.Tanh)
            return hn

        pa_cur = [None]

        def emit_att(t, a_prev, last):
            # 4 col-tiled V-matmuls; step s=t%4 of the group lands on psum
            # partition 32b+s (V sits at relative column s of the window).
            # Rows written by other steps accumulate zeros.
            g, s = divmod(t, GRP)
            if s == 0:
                pa_cur[0] = pap.tile([128, L], F32, name="pa", tag="pa")
            pa = pa_cur[0]
            for b in range(BL):
                nc.tensor.matmul(pa[32 * b:32 * (b + 1), :],
                                 lhsT=v_sb[:, 3 - s:35 - s],
                                 rhs=a_prev[:, L * b:L * (b + 1)],
                                 start=(s == 0), stop=(s == GRP - 1 or last),
                                 tile_position=(0, 32 * b),
                                 skip_group_check=True)
            if s == GRP - 1 or last:
                stg = stgp.tile([128, L], F32, name="stg", tag="stg")
                nc.vector.tensor_copy(stg[:], pa[:])
                if out_dmas:
                    # bounce through DRAM: SBUF->DRAM contiguous, then a
                    # DRAM->DRAM gather of rows {32b+i} -> atts[4g+i, b, :]
                    scr = dscr.tile([128, L], F32, name="scr", tag="scr")
                    nc.sync.dma_start(scr[:, :], stg[:])
                    src = scr.rearrange("(b i) l -> b i l", b=BL)[:, 0:s + 1, :]
                    out = atts[GRP * g:GRP * g + s + 1, :, :].rearrange(
                        "s b l -> b s l")
                    nc.sync.dma_start(out, src)

        if level < 3:
            nc.sync.dma_start(atts[0, 0:1, :], zb[0:1, 0, 0:L])
            return
        h_cur = advance(fh_sb, -1)                      # h(0)
        if level < 4:
            nc.sync.dma_start(atts[0, 0:1, :], zb[0:1, 0, 0:L])
            return
        h_nxt = advance(h_cur, 0)                       # h(1)
        a_prev = None
        # Steady state of iteration t: q(t) from h(t+1); the 4 attention
        # tanhs of step t are split around the (cheap, critical) h(t+2)
        # recurrence ops so ACT never idles and the recurrence never queues
        # behind a full attention burst.  V-matmuls run one step behind so
        # the PE never waits on ACT.
        for t in range(n_steps):
            pq = pqp.tile([128, BL], F32, name="pq", tag="pq")
            for k in range(KC):
                nc.tensor.matmul(pq[:], lhsT=wi_sb[:, D3 * k:D3 * (k + 1)],
                                 rhs=h_nxt[:, k, :],
                                 start=(k == 0), stop=(k == KC - 1))
            # pre-add q (per-b broadcast, read straight from psum) on DVE so
            # the attention tanh is one big ACT op instead of four biased
            # ones; bi is already folded into hdcT
            apre = apool.tile([128, BLL], F32, name="apre", tag="apre")
            for b in range(BL):
                nc.vector.tensor_scalar_add(apre[:, L * b:L * (b + 1)],
                                            hdcT[:, L * b:L * (b + 1)],
                                            pq[:, b:b + 1])
            a_sb = apool.tile([128, BLL], W_DT, name="a_sb", tag="a_sb")
            nc.scalar.activation(a_sb[:, 0:2 * L], apre[:, 0:2 * L], AFT.Tanh)
            h_new = advance(h_nxt, t + 1) if t + 1 < n_steps else None
            nc.scalar.activation(a_sb[:, 2 * L:BLL], apre[:, 2 * L:BLL],
                                 AFT.Tanh)
            if a_prev is not None:
                emit_att(t - 1, a_prev, last=False)
            a_prev = a_sb
            h_cur, h_nxt = h_nxt, h_new
        emit_att(n_steps - 1, a_prev, last=True)
        if not out_dmas:
            nc.sync.dma_start(atts[0, 0:1, :], hdcT[0:1, 0:L])


def make_in_maps(input_ids, first_hidden, context, emb_table, Wi, bi, Wh, bh,
                 V, first_emb, Wd, bd):
    """Host-side sharding + layout prep. All args are np.ndarray (fp32/int)."""
    feats = emb_table[input_ids.astype(np.int64)]        # [B, S, E]
    wdtT = np.ascontiguousarray(Wd[:E, :]).astype(W_NP)
    wdbT = np.ascontiguousarray(Wd[E:, :]).astype(W_NP)
    wiT = Wi.astype(W_NP)
    whT = np.ascontiguousarray(Wh.astype(np.float32))
    vT = V.reshape(D3, 1).astype(W_NP)
    bhT = bh.reshape(D3, 1).astype(np.float32)
    biT = bi.reshape(D3, 1).astype(np.float32)
    bdT = np.ascontiguousarray(bd.reshape(JB, 128).T).astype(np.float32)

    in_maps = []
    for c in range(NCORES):
        b0 = c * BL
        ctxT = np.ascontiguousarray(
            context[b0:b0 + BL].transpose(2, 0, 1).reshape(H, BLL)
        ).astype(np.float32)
        featsT = np.empty((H, NZ + 1), dtype=W_NP)
        # column order: s*BL + b  (so step t's batch columns are contiguous)
        featsT[:, :NZ] = feats[b0:b0 + BL].transpose(2, 1, 0).reshape(H, NZ)
        featsT[:, NZ] = first_emb
        fhT = np.ascontiguousarray(first_hidden[b0:b0 + BL].T).astype(W_NP)
        in_maps.append(dict(ctxT=ctxT, featsT=featsT, fhT=fhT, wdtT=wdtT,
                            wdbT=wdbT, wiT=wiT, whT=whT, vT=vT, bhT=bhT,
                            biT=biT, bdT=bdT))
    return in_maps


_NC_CACHE = []


def kernel(input_ids, mask, first_hidden, context, emb_table, Wi, bi, Wh, bh,
           V, first_emb, Wd, bd):
    input_ids = np.asarray(input_ids)
    first_hidden = np.asarray(first_hidden, dtype=np.float32)
    context = np.asarray(context, dtype=np.float32)
    emb_table = np.asarray(emb_table, dtype=np.float32)
    Wi = np.asarray(Wi, dtype=np.float32)
    bi = np.asarray(bi, dtype=np.float32)
    Wh = np.asarray(Wh, dtype=np.float32)
    bh = np.asarray(bh, dtype=np.float32)
    V = np.asarray(V, dtype=np.float32)
    first_emb = np.asarray(first_emb, dtype=np.float32)
    Wd = np.asarray(Wd, dtype=np.float32)
    bd = np.asarray(bd, dtype=np.float32)

    if not _NC_CACHE:
        _NC_CACHE.append(build_nc())
    nc = _NC_CACHE[0]

    in_maps = make_in_maps(input_ids, first_hidden, context, emb_table, Wi,
                           bi, Wh, bh, V, first_emb, Wd, bd)
    res = run_bass_kernel_spmd(nc, in_maps, core_ids=list(range(NCORES)))

    atts = np.empty((S, B, 1, L), dtype=np.float32)
    for c in range(NCORES):
        atts[:, c * BL:(c + 1) * BL, 0, :] = res.results[c]["atts"]
    prts = np.zeros((S, B), dtype=np.int32)
    return atts, prts
